# revision 1
# baseline (speedup 1.0000x reference)
"""Multi-head GAT layer (PyG GATConv semantics + skip + ELU) on 8 Trainium2 NeuronCores.

v3 strategy (dst-sharded edges, gather-free, bf16):
  - Real edges sorted by dst, tiled 128 edges per dst-block tile (host side).
  - The host ships EDGE-ALIGNED inputs so the device never does an indirect
    gather (HW indirect DMA supports only one index per partition per ~1us
    SWDGE op, which would serialize):
      * xe_T  [256, E_pad]: x[src_e] for every edge slot, transposed bf16
      * stT   [128, E_pad]: per-tile one-hot transpose ST[n, e] = (rel[e]==n)
      * rel2_T: duplicated-pair rel values for the 2x-mode one-hot build
  - Phase 1 (own shard): h_aug = x_own @ [W|A_src|A_dst|skip_W] kept in SBUF
    (a_dst for edge logits, self-loop terms, skip path).
  - Phase 2 per tile (PSUM-bank-aligned groups of KG):
      * hE = xe @ [W|A_src] on the PE -> PSUM [128, 260] (per-edge h | a_src)
      * e = a_src + a_dst via a tiny accumulating matmul: ST.T @ a_dst_blk
        added straight onto hE's a_src columns (no elementwise add)
      * leakyrelu (DVE) ; exp -> duplicated bf16 pairs in ONE Act op
      * msg = h * exp fused with the PSUM->SBUF bf16 cast (DVE pair-trick, or
        per-head Act Copy-with-scale; tiles are split across both engines)
      * one-hot S built per group in one 2x DVE op (host rel pairs vs iota)
      * scatter-add U[:,0:256] += S.T @ msg and U[:,256:260] += S.T @ exp
        (second tiny matmul reads the strided exp pairs directly)
  - Per-block epilogue: self-loop term, divide by denom, skip, ELU, DMA out.
"""

import numpy as np
import ml_dtypes

from concourse import bacc, mybir, tile
from concourse.bass_utils import run_bass_kernel_spmd

P = 128
HEADS = 4
C = 64
HC = HEADS * C          # 256
IN_DIM = 256
EW = HC + HEADS         # 260 per-edge cols: h | a_src(-> e -> exp)
ES = 512                # PSUM stride per tile (2KB bank aligned)
UCOLS = HC + HEADS      # 260 scatter cols: msg | exp
HOWN = HC + 2 * HEADS + HC  # 520 own-shard cols: h | a_src | a_dst | skip
ADST0 = HC + HEADS      # a_dst offset within HOWN
NCORES = 8
NEG_SLOPE = 0.2
EPS = 1e-16
KG = 2                  # tiles per PSUM group
LG = 16                 # tiles per xe/stT load slice (8 groups)
G1 = 8                  # blocks per phase-1 x-load

F32 = mybir.dt.float32
BF16 = mybir.dt.bfloat16
I32 = mybir.dt.int32
AF = mybir.ActivationFunctionType
OP = mybir.AluOpType
NPBF = ml_dtypes.bfloat16


# ----------------------------------------------------------------------------- host prep

def _plan(edge_index: np.ndarray, n_real: int):
    bpc = int(np.ceil(n_real / (NCORES * P)))
    n_pad = NCORES * bpc * P
    nblk = NCORES * bpc

    src = np.ascontiguousarray(edge_index[0]).astype(np.int64)
    dst = np.ascontiguousarray(edge_index[1]).astype(np.int64)

    order = np.argsort(dst, kind="stable")
    s_sorted = src[order].astype(np.int32)
    d_sorted = dst[order].astype(np.int32)

    blk_of_edge = d_sorted >> 7
    counts = np.bincount(blk_of_edge, minlength=nblk)
    starts = np.concatenate([[0], np.cumsum(counts)])

    cnt_cb = counts.reshape(NCORES, bpc)
    tb = np.maximum(1, np.ceil(cnt_cb / P).astype(np.int64)).max(axis=0)
    t_total = int(tb.sum())
    tstart = np.concatenate([[0], np.cumsum(tb)])

    src_T = np.zeros((NCORES, P, t_total), dtype=np.int32)
    rel_T = np.full((NCORES, P, t_total), 255, dtype=np.int32)

    for c in range(NCORES):
        for b in range(bpc):
            g = c * bpc + b
            e0, e1 = starts[g], starts[g + 1]
            cnt = e1 - e0
            ntile = int(tb[b])
            cap = ntile * P
            bs = np.zeros(cap, dtype=np.int32)
            br = np.full(cap, 255, dtype=np.int32)
            bs[:cnt] = s_sorted[e0:e1]
            br[:cnt] = d_sorted[e0:e1] - g * P
            t0 = tstart[b]
            src_T[c, :, t0:t0 + ntile] = bs.reshape(ntile, P).T
            rel_T[c, :, t0:t0 + ntile] = br.reshape(ntile, P).T

    return dict(bpc=bpc, n_pad=n_pad, tb=tb.tolist(), t_total=t_total,
                tstart=tstart.tolist(), src_T=src_T, rel_T=rel_T)


def _weights(W, att_src, att_dst, skip_W):
    w_all = np.zeros((IN_DIM, HOWN), dtype=np.float32)
    w_all[:, 0:HC] = W
    wr = W.reshape(IN_DIM, HEADS, C)
    w_all[:, HC:HC + HEADS] = np.einsum("khc,hc->kh", wr, att_src)
    w_all[:, ADST0:ADST0 + HEADS] = np.einsum("khc,hc->kh", wr, att_dst)
    w_all[:, HOWN - HC:] = skip_W
    return w_all.astype(NPBF)


# ----------------------------------------------------------------------------- program

def _b(ap, n):
    """Broadcast: append 0-stride dim of size n."""
    ap = ap.unsqueeze(ap.ndim)
    return ap.to_broadcast(list(ap.shape[:-1]) + [n])


def build_program(bpc, t_total, tb, tstart):
    rows_c = bpc * P
    e_pad = t_total * P
    nc = bacc.Bacc("TRN2", target_bir_lowering=False, debug=False,
                   num_devices=NCORES)

    xe_T = nc.dram_tensor("xe_T", [IN_DIM, e_pad], BF16, kind="ExternalInput").ap()
    stT = nc.dram_tensor("stT", [P, e_pad], BF16, kind="ExternalInput").ap()
    s3T = nc.dram_tensor("s3T", [P, e_pad], BF16, kind="ExternalInput").ap()
    rel2_T = nc.dram_tensor("rel2_T", [P, 2 * t_total], BF16, kind="ExternalInput").ap()
    xT_own = nc.dram_tensor("xT_own", [IN_DIM, rows_c], BF16, kind="ExternalInput").ap()
    w_all = nc.dram_tensor("w_all", [IN_DIM, HOWN], BF16, kind="ExternalInput").ap()
    out_shard = nc.dram_tensor("out_shard", [rows_c, HC], F32, kind="ExternalOutput").ap()

    blk_of_tile = np.searchsorted(np.asarray(tstart), np.arange(t_total), side="right") - 1

    with tile.TileContext(nc) as tc:
        with (
            tc.tile_pool(name="persist", bufs=1) as persist,
            tc.tile_pool(name="xio", bufs=4) as xio,
            tc.tile_pool(name="xeio", bufs=4) as xeio,
            tc.tile_pool(name="gio", bufs=4) as gio,
            tc.tile_pool(name="eio", bufs=4) as eio,
            tc.tile_pool(name="epi", bufs=3) as epi,
            tc.tile_pool(name="psum_h", bufs=3, space="PSUM") as psum_h,
            tc.tile_pool(name="psum_u", bufs=2, space="PSUM") as psum_u,
        ):
            # ---- persistent SBUF
            h_all = persist.tile([P, bpc * HOWN], BF16)
            rel2_sb = persist.tile([P, 2 * t_total], BF16)
            iota_i = persist.tile([P, P], I32)
            iota16 = persist.tile([P, P], BF16)
            zero_hc = persist.tile([P, HC], F32)
            nc.vector.memset(zero_hc[:], 0.0)
            nc.sync.dma_start(out=rel2_sb[:], in_=rel2_T[:, :])
            nc.gpsimd.iota(iota_i[:], pattern=[[1, P]], base=0, channel_multiplier=0)
            nc.vector.tensor_copy(iota16[:], iota_i[:])

            w_sb = []
            for i in range(2):
                wt = persist.tile([P, HOWN], BF16, name=f"w_sb{i}")
                nc.sync.dma_start(out=wt[:], in_=w_all[i * P:(i + 1) * P, :])
                w_sb.append(wt)

            # ---- phase 1: own-shard h_aug into SBUF
            for g in range((bpc + G1 - 1) // G1):
                kb = min(G1, bpc - g * G1)
                xos = []
                for i in range(2):
                    xo = xio.tile([P, G1 * P], BF16, tag=f"xo{i}", name=f"xo{i}_{g}")
                    nc.sync.dma_start(
                        out=xo[:, 0:kb * P],
                        in_=xT_own[i * P:(i + 1) * P, g * G1 * P:(g * G1 + kb) * P])
                    xos.append(xo)
                for j in range(kb):
                    b = g * G1 + j
                    # reuse the phase-2 PSUM tag/shape so the pool stays at
                    # 2 x 3 banks; hA/hB live in separate banks of one tile
                    hP = psum_h.tile([P, KG * ES], F32, tag="hEg", space="PSUM",
                                     name=f"hP_o{b}")
                    hA = hP[:, 0:HOWN // 2]
                    hB = hP[:, ES:ES + HOWN - HOWN // 2]
                    for i in range(2):
                        nc.tensor.matmul(hA, xos[i][:, j * P:(j + 1) * P],
                                         w_sb[i][:, 0:HOWN // 2],
                                         start=(i == 0), stop=(i == 1))
                    for i in range(2):
                        nc.tensor.matmul(hB, xos[i][:, j * P:(j + 1) * P],
                                         w_sb[i][:, HOWN // 2:HOWN],
                                         start=(i == 0), stop=(i == 1))
                    w0 = b * HOWN
                    nc.scalar.activation(h_all[:, w0:w0 + HOWN // 2], hA, AF.Copy)
                    nc.vector.tensor_copy(h_all[:, w0 + HOWN // 2:w0 + HOWN], hB)

            # ---- phase 2
            n_groups = (t_total + KG - 1) // KG
            u_psum = {}
            xes = None
            sts = None
            s3l = None
            for g in range(n_groups):
                t0 = g * KG
                k = min(KG, t_total - t0)

                if g % (LG // KG) == 0:
                    l0 = t0
                    lk = min(LG, t_total - l0)
                    xes = []
                    for i in range(2):
                        xe = xeio.tile([P, LG * P], BF16, tag=f"xe{i}",
                                       name=f"xe{i}_{g}")
                        nc.sync.dma_start(
                            out=xe[:, 0:lk * P],
                            in_=xe_T[i * P:(i + 1) * P, l0 * P:(l0 + lk) * P])
                        xes.append(xe)
                    sts = xeio.tile([P, LG * P], BF16, tag="st", name=f"st{g}")
                    nc.sync.dma_start(out=sts[:, 0:lk * P],
                                      in_=stT[:, l0 * P:(l0 + lk) * P])
                    s3l = xeio.tile([P, LG * P], BF16, tag="s3l", name=f"s3l{g}")
                    nc.sync.dma_start(out=s3l[:, 0:lk * P],
                                      in_=s3T[:, l0 * P:(l0 + lk) * P])

                hEg = psum_h.tile([P, KG * ES], F32, tag="hEg", space="PSUM",
                                  name=f"hEg{g}")
                gbuf = gio.tile([P, KG * UCOLS], BF16, tag="gbuf", name=f"gbuf{g}")
                e2 = eio.tile([P, KG * HEADS * 2], BF16, tag="e2", name=f"e2{g}")

                for j in range(k):
                    t = t0 + j
                    m = t - (t // LG) * LG
                    he = hEg[:, j * ES:j * ES + EW]
                    for i in range(2):
                        nc.tensor.matmul(he, xes[i][:, m * P:(m + 1) * P],
                                         w_sb[i][:, 0:EW],
                                         start=(i == 0), stop=(i == 1))
                    # e = a_src + a_dst via accumulating 4-col matmul
                    b = int(blk_of_tile[t])
                    w0b = b * HOWN
                    nc.tensor.matmul(hEg[:, j * ES + HC:j * ES + EW],
                                     sts[:, m * P:(m + 1) * P],
                                     h_all[:, w0b + ADST0:w0b + ADST0 + HEADS],
                                     start=False, stop=True, skip_group_check=True)

                h3 = hEg[:].rearrange("p (t f) -> p t f", f=ES)
                ev = h3[:, 0:k, HC:EW]
                # leaky = max(e, 0.2e) in two single-PSUM-read DVE ops
                # (PSUM can feed only one non-scalar input per instruction)
                av = eio.tile([P, KG * HEADS], BF16, tag="av", name=f"av{g}")
                av3 = av[:].rearrange("p (t h) -> p t h", h=HEADS)[:, 0:k, :]
                nc.vector.tensor_scalar(out=av3, in0=ev, scalar1=NEG_SLOPE,
                                        scalar2=None, op0=OP.mult)
                nc.vector.tensor_tensor(out=av3, in0=ev, in1=av3, op=OP.max)
                # exp -> adjacent bf16 pairs, both halves in one Act op
                e2v = e2[:].rearrange("p (t h two) -> p t two h", h=HEADS, two=2)
                nc.scalar.activation(e2v[:, 0:k, :, :],
                                     av3.unsqueeze(2).to_broadcast([P, k, 2, HEADS]),
                                     AF.Exp)

                # PSUM->SBUF bf16 cast of the whole group's h on Act, then a
                # single 2x-mode DVE multiply by the exp pairs; exp itself is
                # dropped into gbuf's trailing 4 cols for the denominator.
                g3 = gbuf[:].rearrange("p (t f) -> p t f", f=UCOLS)
                nc.scalar.activation(g3[:, 0:k, 0:HC], h3[:, 0:k, 0:HC], AF.Copy)
                e2even5 = (e2[:].rearrange("p (t h two) -> p t two h",
                                           h=HEADS, two=2)[:, 0:k, 0:1, :].squeeze(2))
                nc.vector.tensor_copy(g3[:, 0:k, HC:UCOLS], e2even5)
                m5o = (gbuf[:].rearrange("p (t f) -> p t f", f=UCOLS)
                       [:, 0:k, 0:HC]
                       .rearrange("p t (h c2 two) -> p t h c2 two", c2=32, two=2))
                e2p5 = (e2[:].rearrange("p (t h two) -> p t h two",
                                        h=HEADS, two=2)[:, 0:k]
                        .unsqueeze(3).to_broadcast([P, k, HEADS, 32, 2]))
                nc.vector.tensor_tensor(out=m5o, in0=m5o, in1=e2p5, op=OP.mult)

                for j in range(k):
                    t = t0 + j
                    m = t - (t // LG) * LG
                    b = int(blk_of_tile[t])
                    first = (t == tstart[b])
                    last = (t == tstart[b + 1] - 1)
                    if first:
                        u_psum[b] = psum_u.tile([P, UCOLS], F32, tag="u_psum",
                                                space="PSUM", name=f"u{b}")
                    nc.tensor.matmul(
                        u_psum[b][:],
                        lhsT=s3l[:, m * P:(m + 1) * P],
                        rhs=gbuf[:, j * UCOLS:(j + 1) * UCOLS],
                        start=first, stop=last)
                    if last:
                        _epilogue(nc, epi, u_psum.pop(b), h_all, out_shard, b, zero_hc)

    nc.compile()
    return nc


def _epilogue(nc, epi, U, h_all, out_shard, b, zero_hc):
    w0 = b * HOWN
    # self-loop: e_self = a_src_own + a_dst_own -> leaky -> exp
    es = epi.tile([P, HEADS], F32, tag="es", name=f"es{b}")
    nc.vector.tensor_tensor(out=es[:], in0=h_all[:, w0 + HC:w0 + HC + HEADS],
                            in1=h_all[:, w0 + ADST0:w0 + ADST0 + HEADS], op=OP.add)
    nc.vector.scalar_tensor_tensor(out=es[:], in0=es[:], scalar=NEG_SLOPE,
                                   in1=es[:], op0=OP.mult, op1=OP.max)
    nc.scalar.activation(es[:], es[:], AF.Exp)
    # denom = U[:,256:260] + exp_self + eps ; r = 1/denom
    r4 = epi.tile([P, HEADS], F32, tag="r4", name=f"r4{b}")
    nc.vector.scalar_tensor_tensor(out=r4[:], in0=U[:, HC:UCOLS], scalar=EPS,
                                   in1=es[:], op0=OP.add, op1=OP.add)
    nc.vector.reciprocal(r4[:], r4[:])
    # pre = (U + h_own*exp_self) * r + skip
    pre = epi.tile([P, HC], F32, tag="pre", name=f"pre{b}")
    p4 = pre[:].rearrange("p (h c) -> p h c", c=C)
    h4 = h_all[:, w0:w0 + HC].rearrange("p (h c) -> p h c", c=C)
    nc.vector.tensor_tensor(out=p4, in0=h4, in1=_b(es[:], C), op=OP.mult)
    nc.vector.tensor_tensor(out=pre[:], in0=pre[:], in1=U[:, 0:HC], op=OP.add)
    nc.vector.tensor_tensor(out=p4, in0=p4, in1=_b(r4[:], C), op=OP.mult)
    nc.gpsimd.tensor_tensor(out=pre[:], in0=pre[:],
                            in1=h_all[:, w0 + HOWN - HC:w0 + HOWN], op=OP.add)
    # ELU(x) = max(x, exp(min(x,0)) - 1)
    m = epi.tile([P, HC], F32, tag="m", name=f"m{b}")
    nc.vector.tensor_scalar(out=m[:], in0=pre[:], scalar1=0.0, scalar2=None,
                            op0=OP.min)
    nc.scalar.activation(m[:], m[:], AF.Exp)
    ob = epi.tile([P, HC], F32, tag="ob", name=f"ob{b}")
    nc.vector.scalar_tensor_tensor(out=ob[:], in0=m[:], scalar=-1.0, in1=pre[:],
                                   op0=OP.add, op1=OP.max)
    nc.sync.dma_start(out=out_shard[b * P:(b + 1) * P, :], in_=ob[:])


# ----------------------------------------------------------------------------- driver

_CACHE = {}


def _run(x, edge_index, W, att_src, att_dst, bias, skip_W, trace=False):
    n_real = x.shape[0]
    plan = _plan(np.asarray(edge_index), n_real)
    bpc, n_pad, t_total = plan["bpc"], plan["n_pad"], plan["t_total"]
    assert not np.any(np.asarray(bias)), "bias path not implemented (zeros expected)"
    w_np = _weights(np.asarray(W, np.float32), np.asarray(att_src, np.float32),
                    np.asarray(att_dst, np.float32), np.asarray(skip_W, np.float32))

    key = (n_real, bpc, t_total, tuple(plan["tb"]))
    if key not in _CACHE:
        _CACHE[key] = build_program(bpc, t_total, plan["tb"], plan["tstart"])
    nc = _CACHE[key]

    rows_c = bpc * P
    e_pad = t_total * P
    x_pad = np.zeros((n_pad, IN_DIM), dtype=np.float32)
    x_pad[:n_real] = np.asarray(x, np.float32)
    xT = np.ascontiguousarray(x_pad.T.astype(NPBF))

    in_maps = []
    cols = np.arange(e_pad)
    for c in range(NCORES):
        src_cols = plan["src_T"][c].T.ravel()          # q = t*128+p -> src id
        xe_T = np.ascontiguousarray(xT[:, src_cols])
        rel_flat = plan["rel_T"][c].T.ravel()
        stT = np.zeros((P, e_pad), dtype=NPBF)
        valid = rel_flat < P
        stT[rel_flat[valid], cols[valid]] = 1.0
        # S[e, n] one-hots, edge-partitioned: s3T[p, t*128+n] = (rel[p,t]==n)
        s3T = np.zeros((P, e_pad), dtype=NPBF)
        relc = plan["rel_T"][c]
        pp, tt = np.nonzero(relc < P)
        s3T[pp, tt * P + relc[pp, tt]] = 1.0
        rel2 = np.repeat(plan["rel_T"][c].astype(np.float32), 2, axis=1)
        r2 = np.empty((P, 2 * t_total), dtype=NPBF)
        r2[:, 0::2] = plan["rel_T"][c].astype(NPBF)
        r2[:, 1::2] = plan["rel_T"][c].astype(NPBF)
        in_maps.append(dict(
            xe_T=xe_T,
            stT=stT,
            s3T=s3T,
            rel2_T=r2,
            xT_own=np.ascontiguousarray(xT[:, c * rows_c:(c + 1) * rows_c]),
            w_all=w_np,
        ))

    res = run_bass_kernel_spmd(nc, in_maps, list(range(NCORES)), trace=trace)
    out = np.concatenate([res.results[c]["out_shard"] for c in range(NCORES)], axis=0)
    return out[:n_real], res


def kernel(x, edge_index, W, att_src, att_dst, bias, skip_W):
    out, _ = _run(x, edge_index, W, att_src, att_dst, bias, skip_W, trace=False)
    return out


def build_for_sim(edge_index, n_real):
    plan = _plan(np.asarray(edge_index), n_real)
    return build_program(plan["bpc"], plan["t_total"], plan["tb"], plan["tstart"])



# revision 4
# speedup vs baseline: 1.3434x; 1.3434x over previous
"""Multi-head GAT layer (PyG GATConv semantics + skip + ELU) on 8 Trainium2 NeuronCores.

v4 strategy (dst-sharded edges, fp8 DoubleRow, engine-balanced):
  - Self-loops appended as regular edges on host; edges sorted by dst and
    tiled 128/dst-block tile. Host ships edge-aligned gathers (no device
    indirect DMA):
      * xe8  [128, t*256]: x[src_e] fp8 in DoubleRow k-pair layout [t, 2, 128]
      * stT8 [128, t*128]: one-hot (node-partitioned) for the a_dst gather
      * s3T8 [128, t*128]: one-hot (edge-partitioned) for the scatter-add
  - Phase 1 (own shard): h_aug = x_own @ [W@att_dst | skip_W] kept in SBUF
    bf16 (a_dst for edge logits + skip path); a_dst also cast to fp8.
  - Phase 2, per 16-tile window: per-edge logits a_src (fp8 DR matmul) +
    a_dst (one-hot fp8 matmul) accumulate in a dedicated PSUM bank; one Act
    Prelu + one Act Exp produce fp8 exp factors for the whole window.
  - Per tile: h = xe8 DR-matmul w8 -> PSUM [128,256] (128 cycles).
  - Per KG-tile group: msg = fp8(h * exp) via EITHER one fused DVE op
    (PSUM f32 x fp8-broadcast -> fp8) or Act copy (->bf16) + Pool multiply,
    alternating to balance the three engines.
  - Scatter: U[:,0:256] += s3l8.T DR-matmul msg (two tiles per matmul);
    U[:,256:260] += s3l8.T DR-matmul exp (denominator). Same fp8 exp values
    in msg and denom keep the softmax a convex combination.
  - Per-block epilogue: r=1/denom, U*r, +skip, ELU (Act does the exp branch
    via exp(-relu(-x))), DMA out.
"""

import numpy as np
import ml_dtypes

from concourse import bacc, mybir, tile
from concourse.bass_utils import run_bass_kernel_spmd

P = 128
HEADS = 4
C = 64
HC = HEADS * C          # 256
IN_DIM = 256
HOWN = HEADS + HC       # 260 phase-1 cols: a_dst | skip
UCOLS = HC + HEADS      # 260 gbuf/U cols: msg | exp
NCORES = 8
NEG_SLOPE = 0.2
KG = 4                  # tiles per group (h psum: KG*256 f32 = 2 banks)
LG = 32                 # tiles per load slice
WIN = 16                # tiles per logit window (one PSUM bank of [128,64])
G1 = 8                  # blocks per phase-1 x-load
PATH_B_NUM = 2          # of every PATH_MOD groups, this many use Act+Pool
PATH_MOD = 5

F32 = mybir.dt.float32
BF16 = mybir.dt.bfloat16
FP8 = mybir.dt.float8e4
AF = mybir.ActivationFunctionType
OP = mybir.AluOpType
DR = mybir.MatmulPerfMode.DoubleRow
NPBF = ml_dtypes.bfloat16
NP8 = ml_dtypes.float8_e4m3


# ----------------------------------------------------------------------------- host prep

def _plan(edge_index: np.ndarray, n_real: int):
    bpc = int(np.ceil(n_real / (NCORES * P)))
    n_pad = NCORES * bpc * P
    nblk = NCORES * bpc

    # self-loops as regular edges
    loops = np.arange(n_real, dtype=np.int64)
    src = np.concatenate([np.ascontiguousarray(edge_index[0]).astype(np.int64), loops])
    dst = np.concatenate([np.ascontiguousarray(edge_index[1]).astype(np.int64), loops])

    order = np.argsort(dst, kind="stable")
    s_sorted = src[order].astype(np.int32)
    d_sorted = dst[order].astype(np.int32)

    blk_of_edge = d_sorted >> 7
    counts = np.bincount(blk_of_edge, minlength=nblk)
    starts = np.concatenate([[0], np.cumsum(counts)])

    cnt_cb = counts.reshape(NCORES, bpc)
    tb = np.maximum(1, np.ceil(cnt_cb / P).astype(np.int64)).max(axis=0)
    t_total = int(tb.sum())
    tstart = np.concatenate([[0], np.cumsum(tb)])

    src_T = np.zeros((NCORES, t_total, P), dtype=np.int32)   # [core, tile, slot]
    rel_T = np.full((NCORES, t_total, P), 255, dtype=np.int32)

    for c in range(NCORES):
        for b in range(bpc):
            g = c * bpc + b
            e0, e1 = starts[g], starts[g + 1]
            cnt = e1 - e0
            ntile = int(tb[b])
            cap = ntile * P
            bs = np.zeros(cap, dtype=np.int32)
            br = np.full(cap, 255, dtype=np.int32)
            bs[:cnt] = s_sorted[e0:e1]
            br[:cnt] = d_sorted[e0:e1] - g * P
            t0 = tstart[b]
            src_T[c, t0:t0 + ntile] = bs.reshape(ntile, P)
            rel_T[c, t0:t0 + ntile] = br.reshape(ntile, P)

    return dict(bpc=bpc, n_pad=n_pad, tb=tb.tolist(), t_total=t_total,
                tstart=tstart.tolist(), src_T=src_T, rel_T=rel_T)


def _weights(W, att_src, att_dst, skip_W):
    wr = W.reshape(IN_DIM, HEADS, C)
    # phase-1 bf16: [a_dst | skip]
    w_all = np.zeros((IN_DIM, HOWN), dtype=np.float32)
    w_all[:, 0:HEADS] = np.einsum("khc,hc->kh", wr, att_dst)
    w_all[:, HEADS:] = skip_W
    # phase-2 fp8 DR layout [128, 2, 260]: W | W@att_src
    w_aug = np.zeros((IN_DIM, HC + HEADS), dtype=np.float32)
    w_aug[:, 0:HC] = W
    w_aug[:, HC:] = np.einsum("khc,hc->kh", wr, att_src)
    w8 = np.ascontiguousarray(
        w_aug.reshape(2, P, HC + HEADS).transpose(1, 0, 2)).astype(NP8)
    return w_all.astype(NPBF), w8


# ----------------------------------------------------------------------------- program

def build_program(bpc, t_total, tb, tstart):
    rows_c = bpc * P
    nc = bacc.Bacc("TRN2", target_bir_lowering=False, debug=False,
                   num_devices=NCORES)

    xe8 = nc.dram_tensor("xe8", [P, t_total * 2 * P], FP8, kind="ExternalInput").ap()
    stT8 = nc.dram_tensor("stT8", [P, t_total * P], FP8, kind="ExternalInput").ap()
    s3T8 = nc.dram_tensor("s3T8", [P, t_total * P], FP8, kind="ExternalInput").ap()
    xT_own = nc.dram_tensor("xT_own", [IN_DIM, rows_c], BF16, kind="ExternalInput").ap()
    w_all = nc.dram_tensor("w_all", [IN_DIM, HOWN], BF16, kind="ExternalInput").ap()
    w8d = nc.dram_tensor("w8d", [P, 2 * (HC + HEADS)], FP8, kind="ExternalInput").ap()
    out_shard = nc.dram_tensor("out_shard", [rows_c, HC], F32, kind="ExternalOutput").ap()

    blk_of_tile = np.searchsorted(np.asarray(tstart), np.arange(t_total),
                                  side="right") - 1

    with tile.TileContext(nc) as tc:
        with (
            tc.tile_pool(name="persist", bufs=1) as persist,
            tc.tile_pool(name="xio", bufs=4) as xio,
            tc.tile_pool(name="xeio", bufs=2) as xeio,
            tc.tile_pool(name="gio", bufs=3) as gio,
            tc.tile_pool(name="eio", bufs=3) as eio,
            tc.tile_pool(name="epi", bufs=2) as epi,
            tc.tile_pool(name="psum_h", bufs=2, space="PSUM") as psum_h,
            tc.tile_pool(name="psum_lg", bufs=2, space="PSUM") as psum_lg,
            tc.tile_pool(name="psum_u", bufs=2, space="PSUM") as psum_u,
        ):
            # ---- persistent SBUF
            h_all = persist.tile([P, bpc * HOWN], BF16)
            a_dst8 = persist.tile([P, bpc * HEADS], FP8)
            w8 = persist.tile([P, 2 * (HC + HEADS)], FP8)
            nc.sync.dma_start(out=w8[:], in_=w8d[:, :])
            w83 = w8[:].rearrange("p (i c) -> p i c", i=2)
            w_sb = []
            for i in range(2):
                wt = persist.tile([P, HOWN], BF16, name=f"w_sb{i}")
                nc.sync.dma_start(out=wt[:], in_=w_all[i * P:(i + 1) * P, :])
                w_sb.append(wt)

            # ---- phase 1: own-shard [a_dst | skip] into SBUF
            for g in range((bpc + G1 - 1) // G1):
                kb = min(G1, bpc - g * G1)
                xos = []
                for i in range(2):
                    xo = xio.tile([P, G1 * P], BF16, tag=f"xo{i}", name=f"xo{i}_{g}")
                    nc.sync.dma_start(
                        out=xo[:, 0:kb * P],
                        in_=xT_own[i * P:(i + 1) * P, g * G1 * P:(g * G1 + kb) * P])
                    xos.append(xo)
                for j in range(kb):
                    b = g * G1 + j
                    hP = psum_h.tile([P, KG * HC], F32, tag="hps", space="PSUM",
                                     name=f"hP_o{b}")
                    hv = hP[:, 0:HOWN]
                    for i in range(2):
                        nc.tensor.matmul(hv, xos[i][:, j * P:(j + 1) * P],
                                         w_sb[i][:], start=(i == 0), stop=(i == 1))
                    w0 = b * HOWN
                    nc.scalar.activation(h_all[:, w0:w0 + 130], hP[:, 0:130], AF.Copy)
                    nc.vector.tensor_copy(h_all[:, w0 + 130:w0 + HOWN], hP[:, 130:HOWN])
                # fp8 a_dst for the logit matmuls of these blocks
                ad_src = (h_all[:, g * G1 * HOWN:(g * G1 + kb) * HOWN]
                          .rearrange("p (b f) -> p b f", f=HOWN)[:, :, 0:HEADS])
                ad_dst = (a_dst8[:, g * G1 * HEADS:(g * G1 + kb) * HEADS]
                          .rearrange("p (b h) -> p b h", h=HEADS))
                nc.scalar.activation(ad_dst, ad_src, AF.Copy)

            # ---- phase 2
            n_groups = (t_total + KG - 1) // KG
            u_psum = {}
            xes = sts = s3l = None
            lgp = avt = ex8 = None
            l0 = w0t = 0
            for g in range(n_groups):
                t0 = g * KG
                k = min(KG, t_total - t0)

                if t0 % LG == 0:
                    l0 = t0
                    lk = min(LG, t_total - l0)
                    xes = xeio.tile([P, LG * 2 * P], FP8, tag="xe", name=f"xe{g}")
                    nc.sync.dma_start(out=xes[:, 0:lk * 2 * P],
                                      in_=xe8[:, l0 * 2 * P:(l0 + lk) * 2 * P])
                    sts = xeio.tile([P, LG * P], FP8, tag="st", name=f"st{g}")
                    nc.sync.dma_start(out=sts[:, 0:lk * P],
                                      in_=stT8[:, l0 * P:(l0 + lk) * P])
                    s3l = xeio.tile([P, LG * P], FP8, tag="s3l", name=f"s3l{g}")
                    nc.sync.dma_start(out=s3l[:, 0:lk * P],
                                      in_=s3T8[:, l0 * P:(l0 + lk) * P])
                xe4 = xes[:].rearrange("p (t i e) -> p t i e", i=2, e=P)

                if t0 % WIN == 0:
                    # logits for the whole window, then Prelu+Exp on Act
                    w0t = t0
                    wk = min(WIN, t_total - w0t)
                    lgp = psum_lg.tile([P, WIN * HEADS], F32, tag="lg",
                                       space="PSUM", name=f"lg{g}")
                    for jj in range(wk):
                        t = w0t + jj
                        m = t - l0
                        b = int(blk_of_tile[t])
                        lslice = lgp[:, jj * HEADS:(jj + 1) * HEADS]
                        nc.tensor.matmul(lslice, xe4[:, m], w83[:, :, HC:],
                                         start=True, stop=False, perf_mode=DR)
                        nc.tensor.matmul(lslice, sts[:, m * P:(m + 1) * P],
                                         a_dst8[:, b * HEADS:(b + 1) * HEADS],
                                         start=False, stop=True,
                                         skip_group_check=True)
                    avt = eio.tile([P, WIN * HEADS], BF16, tag="av", name=f"av{g}")
                    nc.scalar.activation(avt[:, 0:wk * HEADS],
                                         lgp[:, 0:wk * HEADS], AF.Prelu,
                                         alpha=NEG_SLOPE)

                # per-edge h via fp8 DoubleRow
                hEg = psum_h.tile([P, KG * HC], F32, tag="hps", space="PSUM",
                                  name=f"hEg{g}")
                for j in range(k):
                    m = t0 + j - l0
                    nc.tensor.matmul(hEg[:, j * HC:(j + 1) * HC], xe4[:, m],
                                     w83[:, :, 0:HC], start=True, stop=True,
                                     perf_mode=DR)

                # gbuf tile layout [t, msg(256) | exp(4)]; exp written by Act,
                # msg = fp8(h * exp) with exp read back from gbuf's own cols.
                gbuf = gio.tile([P, KG * UCOLS], FP8, tag="gbuf", name=f"gbuf{g}")
                gt = gbuf[:].rearrange("p (t f) -> p t f", f=UCOLS)
                av3 = avt[:].rearrange("p (t h) -> p t h", h=HEADS)
                nc.scalar.activation(gt[:, 0:k, HC:], av3[:, t0 - w0t:t0 - w0t + k],
                                     AF.Exp)
                h4 = hEg[:].rearrange("p (t h c) -> p t h c", h=HEADS, c=C)
                g4 = (gt[:, :, 0:HC]
                      .rearrange("p t (h c) -> p t h c", c=C))
                e4 = (gt[:, :, HC:].unsqueeze(3)
                      .to_broadcast([P, KG, HEADS, C]))
                if g % PATH_MOD < PATH_B_NUM:
                    hbf = gio.tile([P, KG * HC], BF16, tag="hbf", name=f"hbf{g}")
                    nc.scalar.activation(hbf[:, 0:k * HC], hEg[:, 0:k * HC], AF.Copy)
                    hb4 = hbf[:].rearrange("p (t h c) -> p t h c", h=HEADS, c=C)
                    nc.gpsimd.tensor_tensor(out=g4[:, 0:k], in0=hb4[:, 0:k],
                                            in1=e4[:, 0:k], op=OP.mult)
                else:
                    nc.vector.tensor_tensor(out=g4[:, 0:k], in0=h4[:, 0:k],
                                            in1=e4[:, 0:k], op=OP.mult)

                # scatter msg+exp into U (DoubleRow pairs within block runs)
                j = 0
                while j < k:
                    t = t0 + j
                    b = int(blk_of_tile[t])
                    if t == tstart[b]:
                        u_psum[b] = psum_u.tile([P, UCOLS], F32, tag="u",
                                                space="PSUM", name=f"u{b}")
                    U = u_psum[b]
                    first = (t == tstart[b])
                    pair = (j + 1 < k and int(blk_of_tile[t + 1]) == b)
                    m = t - l0
                    if pair:
                        last = (t + 1 == tstart[b + 1] - 1)
                        s3p = (s3l[:, m * P:(m + 2) * P]
                               .rearrange("p (i e) -> p i e", i=2))
                        gp = (gbuf[:, j * UCOLS:(j + 2) * UCOLS]
                              .rearrange("p (i c) -> p i c", i=2))
                        nc.tensor.matmul(U[:], s3p, gp, start=first,
                                         stop=last, perf_mode=DR)
                        j += 2
                        t += 1
                    else:
                        last = (t == tstart[b + 1] - 1)
                        nc.tensor.matmul(U[:], s3l[:, m * P:(m + 1) * P],
                                         gbuf[:, j * UCOLS:(j + 1) * UCOLS],
                                         start=first, stop=last)
                        j += 1
                    if t == tstart[b + 1] - 1:
                        _epilogue(nc, epi, u_psum.pop(b), h_all, out_shard, b)

    nc.compile()
    return nc


def _epilogue(nc, epi, U, h_all, out_shard, b):
    w0 = b * HOWN
    # r = 1/denom (self-loop guarantees denom > 0; eps dropped)
    r4 = epi.tile([P, HEADS], F32, tag="r4", name=f"r4{b}")
    nc.vector.reciprocal(r4[:], U[:, HC:])
    # pre = U * r  (DVE), += skip (Pool)
    pre = epi.tile([P, HC], F32, tag="pre", name=f"pre{b}")
    p4 = pre[:].rearrange("p (h c) -> p h c", c=C)
    u4 = U[:, 0:HC].rearrange("p (h c) -> p h c", c=C)
    r4b = r4[:].unsqueeze(2).to_broadcast([P, HEADS, C])
    nc.vector.tensor_tensor(out=p4, in0=u4, in1=r4b, op=OP.mult)
    nc.gpsimd.tensor_tensor(out=pre[:], in0=pre[:],
                            in1=h_all[:, w0 + HEADS:w0 + HOWN], op=OP.add)
    # ELU: exp(min(x,0)) = exp(-relu(-x)) on Act; final max on DVE
    y = epi.tile([P, HC], F32, tag="y", name=f"y{b}")
    nc.scalar.activation(y[:], pre[:], AF.Relu, scale=-1.0)
    m = epi.tile([P, HC], F32, tag="m", name=f"m{b}")
    nc.scalar.activation(m[:], y[:], AF.Exp, scale=-1.0)
    ob = epi.tile([P, HC], F32, tag="ob", name=f"ob{b}")
    nc.vector.scalar_tensor_tensor(out=ob[:], in0=m[:], scalar=-1.0, in1=pre[:],
                                   op0=OP.add, op1=OP.max)
    nc.sync.dma_start(out=out_shard[b * P:(b + 1) * P, :], in_=ob[:])


# ----------------------------------------------------------------------------- driver

_CACHE = {}


def _run(x, edge_index, W, att_src, att_dst, bias, skip_W, trace=False):
    n_real = x.shape[0]
    plan = _plan(np.asarray(edge_index), n_real)
    bpc, n_pad, t_total = plan["bpc"], plan["n_pad"], plan["t_total"]
    assert not np.any(np.asarray(bias)), "bias path not implemented (zeros expected)"
    w_np, w8_np = _weights(np.asarray(W, np.float32), np.asarray(att_src, np.float32),
                           np.asarray(att_dst, np.float32),
                           np.asarray(skip_W, np.float32))

    key = (n_real, bpc, t_total, tuple(plan["tb"]))
    if key not in _CACHE:
        _CACHE[key] = build_program(bpc, t_total, plan["tb"], plan["tstart"])
    nc = _CACHE[key]

    rows_c = bpc * P
    x_pad = np.zeros((n_pad, IN_DIM), dtype=np.float32)
    x_pad[:n_real] = np.asarray(x, np.float32)
    xT8 = np.ascontiguousarray(x_pad.T.astype(NP8))      # [256, n_pad]
    xTb = np.ascontiguousarray(x_pad.T.astype(NPBF))

    in_maps = []
    for c in range(NCORES):
        src_c = plan["src_T"][c]                          # [t_total, 128]
        # xe8: [128p, t, 2, 128e]
        ge = xT8[:, src_c.ravel()]                        # [256, t*128]
        xe = np.ascontiguousarray(
            ge.reshape(2, P, t_total, P).transpose(1, 2, 0, 3)
        ).reshape(P, t_total * 2 * P)
        rel_c = plan["rel_T"][c]                          # [t_total, 128]
        valid = rel_c < P
        tt, ee = np.nonzero(valid)
        st = np.zeros((P, t_total * P), dtype=NP8)        # [n, t*128+e]
        st[rel_c[tt, ee], tt * P + ee] = 1.0
        s3 = np.zeros((P, t_total * P), dtype=NP8)        # [e, t*128+n]
        s3[ee, tt * P + rel_c[tt, ee]] = 1.0
        in_maps.append(dict(
            xe8=xe, stT8=st, s3T8=s3,
            xT_own=np.ascontiguousarray(xTb[:, c * rows_c:(c + 1) * rows_c]),
            w_all=w_np, w8d=w8_np,
        ))

    res = run_bass_kernel_spmd(nc, in_maps, list(range(NCORES)), trace=trace)
    out = np.concatenate([res.results[c]["out_shard"] for c in range(NCORES)], axis=0)
    return out[:n_real], res


def kernel(x, edge_index, W, att_src, att_dst, bias, skip_W):
    out, _ = _run(x, edge_index, W, att_src, att_dst, bias, skip_W, trace=False)
    return out


def build_for_sim(edge_index, n_real):
    plan = _plan(np.asarray(edge_index), n_real)
    return build_program(plan["bpc"], plan["t_total"], plan["tb"], plan["tstart"])
